# revision 1
# baseline (speedup 1.0000x reference)
"""Trainium2 Bass kernel for a grouped contrastive loss.

Math (matches the reference):
    z_a = concat(z_target, z_source)                      # [A=M+N, D]
    sims[a, j] = (z_a[a] . z_target[j]) / T
    den[j]  = sum_a exp(sims[a, j]) - exp(z_tj.z_tj / T)
    num[j]  = mean_{s: seg_source[s]==seg_target[j]} (z_s . z_tj) / T
            = (v_j . z_tj)   with v_j = S[seg_target[j]] / (count * T),
              S[g] = sum of z_source rows in group g       (exact linearity)
    loss = sum_j log(den[j]) - num[j]

Sharding: target columns j split across 8 cores (512 each); z_a replicated
(per the sharding hint). Per core, per 128-column block jb and 2048-wide
a-range g: PE matmuls (fp16 in, fp32 accum) fill a 4-bank PSUM tile, and one
ScalarE Exp activation with accum_out produces the per-column partial
exp-sums directly (cols of `res`). DVE computes the numerator partial
sum_j v'_j . z_tj (res2). ACT is the bottleneck engine (~1 elem/cycle/lane
for 33.5M exps across 8 cores); matmuls and DMAs hide underneath it.

The self term exp(z_tj.z_tj/T) ~ exp(1/T) ~ 1.6e6 dwarfs den ~ 1.8e4, so the
host must subtract (nearly) exactly what the device folded into the exp sums:
with fp16 inputs each PE product is exact in fp32, and np.sum's fp32 pairwise
accumulation reproduces the PE accumulator to ~2 ulp (verified on hardware),
which keeps the cancellation residual at ~1e-5 of the loss.

Host: tiny final reduction (log over 4096 columns + scalar sums) in float64.
"""

import numpy as np

TEMPERATURE = 0.07
N = 4096       # z_source rows
M = 4096       # z_target rows
D = 128        # embedding dim
G = 64         # groups
NCORES = 8
MLOC = M // NCORES          # 512 target columns per core
A = M + N                   # 8192 rows of z_a
ACH = 512                   # a-chunk (one matmul rhs / one PSUM bank)
NCH = A // ACH              # 16 chunks
GRP = 4                     # chunks per ACT group -> 2048-wide exp+accum
NGRP = NCH // GRP           # 4 groups per jb
NJB = MLOC // 128           # 4 column blocks of 128 per core
SPLITS = {}   # ACT subdivision (was used to absorb early DMA bubbles; now stale)


def _layout():
    """Each group's exp row-sum goes to one of two sinks: ACT accum_out
    (res, odd groups — includes the final group so the tail stays short) or
    a DVE tensor_reduce (res2 cols >= 1, even groups — saves the 187ns ACT
    accumulator-read per group). res2 is written only by DVE; res only by
    ACT: no cross-engine tile sharing."""
    acc_cols = {}
    dve_cols = {}
    ca, cd = 0, 1          # res2 col 0 = num partial
    dve_set = {0, 1, 2, 3, 4, 6, 7, 8, 9, 10, 12, 13}   # not 14/15: DVE must
    for jb in range(NJB):                          # finish before ACT does
        for g in range(NGRP):
            n = SPLITS.get((jb, g), 1)
            if jb * NGRP + g in dve_set:
                dve_cols[(jb, g)] = (cd, n)
                cd += n
            else:
                acc_cols[(jb, g)] = (ca, n)
                ca += n
    return acc_cols, dve_cols, ca, cd


_ACC_COLS, _DVE_COLS, NSUM, RES2_COLS = _layout()

_CACHE = {}


def _build_bass():
    import concourse.mybir as mybir
    from concourse import bacc
    from concourse.tile import TileContext

    f32 = mybir.dt.float32
    f32r = mybir.dt.float32r
    f16 = mybir.dt.float16

    nc = bacc.Bacc("TRN2", num_devices=NCORES)
    zaT = nc.dram_tensor("zaT", [D, A], f16, kind="ExternalInput")
    ztT = nc.dram_tensor("ztT", [D, MLOC], f16, kind="ExternalInput")
    vtT = nc.dram_tensor("vtT", [D, MLOC], f32, kind="ExternalInput")
    res = nc.dram_tensor("res", [128, NSUM], f32, kind="ExternalOutput")
    res2 = nc.dram_tensor("res2", [128, RES2_COLS], f32, kind="ExternalOutput")

    with TileContext(nc) as tc:
        with (
            tc.tile_pool(name="persist", bufs=1) as persist,
            tc.tile_pool(name="scratch", bufs=2) as scratch,
            tc.tile_pool(name="scratch3", bufs=4) as scratch3,
            tc.tile_pool(name="psum", bufs=2, space="PSUM") as psum_pool,
        ):
            # DMA order follows the critical chain: the jb=0 weight slice
            # (32KB) lands first, then the first matmul group's rhs, then the
            # rest. HWDGE issue cost is ~constant per DMA, so few big
            # transfers win over per-chunk loads.
            zt_tile = persist.tile([128, MLOC], f16, tag="zt")
            nc.sync.dma_start(out=zt_tile[:, 0:128], in_=ztT[:, 0:128])
            za_tiles = []
            t0 = persist.tile([128, GRP * ACH], f16, tag="za0")
            nc.sync.dma_start(out=t0[:, 0:1024], in_=zaT[:, 0:1024])
            nc.sync.dma_start(out=t0[:, 1024:2048], in_=zaT[:, 1024:2048])
            za_tiles.append(t0)
            res2_tile = persist.tile([128, RES2_COLS], f32, tag="res2")
            t1 = persist.tile([128, GRP * ACH], f16, tag="za1")
            nc.sync.dma_start(out=t1[:, 0:1024], in_=zaT[:, 2048:3072])
            nc.sync.dma_start(out=t1[:, 1024:2048], in_=zaT[:, 3072:4096])
            za_tiles.append(t1)
            for g in range(2, NGRP):
                t = persist.tile([128, GRP * ACH], f16, tag=f"za{g}")
                nc.sync.dma_start(
                    out=t[:], in_=zaT[:, g * GRP * ACH:(g + 1) * GRP * ACH]
                )
                za_tiles.append(t)
            # zt columns beyond jb=0 are first consumed ~15us in; load last
            nc.sync.dma_start(out=zt_tile[:, 128:MLOC], in_=ztT[:, 128:MLOC])
            res_tile = persist.tile([128, NSUM], f32, tag="res")

            def emit_group(jb, g):
                use_dve = (jb, g) in _DVE_COLS
                sumcol, nh = (_DVE_COLS if use_dve else _ACC_COLS)[(jb, g)]
                lhsT = zt_tile[:, jb * 128:(jb + 1) * 128]
                ps = psum_pool.tile([128, GRP * ACH], f32, tag="ps")
                for k in range(GRP):
                    nc.tensor.matmul(
                        ps[:, k * ACH:(k + 1) * ACH],
                        lhsT,
                        za_tiles[g][:, k * ACH:(k + 1) * ACH],
                        start=True,
                        stop=True,
                    )
                pool = scratch3 if use_dve else scratch
                scr = pool.tile([128, GRP * ACH], f32,
                                tag="expscrD" if use_dve else "expscr")
                # Early groups' ACT is subdivided so the exp stream starts
                # before the whole first rhs region has arrived.
                w = GRP * ACH // nh
                for h in range(nh):
                    nc.scalar.activation(
                        out=scr[:, h * w:(h + 1) * w],
                        in_=ps[:, h * w:(h + 1) * w],
                        func=mybir.ActivationFunctionType.Exp,
                        scale=1.0 / TEMPERATURE,
                        accum_out=None if use_dve
                        else res_tile[:, sumcol + h:sumcol + h + 1],
                    )
                    if use_dve:
                        nc.vector.tensor_reduce(
                            out=res2_tile[:, sumcol + h:sumcol + h + 1],
                            in_=scr[:, h * w:(h + 1) * w],
                            axis=mybir.AxisListType.X,
                            op=mybir.AluOpType.add,
                        )

            emit_group(0, 0)

            # num partial (independent; emitted early so its DMA + DVE work
            # happen in the shadow of the exp stream): sum_j (v'_j . z_tj)
            # reduced along the free axis; partition (D) axis summed on host.
            vt_tile = persist.tile([128, MLOC], f32, tag="vt")
            nc.sync.dma_start(out=vt_tile[:], in_=vtT[:, :])
            zt_f32 = scratch.tile([128, MLOC], f32, tag="ztf32")
            nc.vector.tensor_copy(out=zt_f32[:], in_=zt_tile[:])
            num_scr = scratch.tile([128, MLOC], f32, tag="numscr")
            nc.vector.tensor_mul(out=num_scr[:], in0=vt_tile[:], in1=zt_f32[:])
            nc.vector.tensor_reduce(
                out=res2_tile[:, 0:1],
                in_=num_scr[:],
                axis=mybir.AxisListType.X,
                op=mybir.AluOpType.add,
            )

            for jb in range(NJB):
                for g in range(NGRP):
                    if jb == 0 and g == 0:
                        continue
                    emit_group(jb, g)

            nc.sync.dma_start(out=res2[:, :], in_=res2_tile[:])
            if NSUM > 2:
                nc.sync.dma_start(out=res[:, :NSUM - 2], in_=res_tile[:, :NSUM - 2])
                nc.sync.dma_start(out=res[:, NSUM - 2:], in_=res_tile[:, NSUM - 2:])
            else:
                nc.sync.dma_start(out=res[:, :], in_=res_tile[:])
    nc.compile()
    return nc


def _get_nc():
    if "nc" not in _CACHE:
        _CACHE["nc"] = _build_bass()
    return _CACHE["nc"]


def kernel(z_source, z_target, seg_source, seg_target):
    from concourse.bass_utils import run_bass_kernel_spmd

    zs = np.ascontiguousarray(z_source, dtype=np.float32)
    zt = np.ascontiguousarray(z_target, dtype=np.float32)
    seg_s = np.asarray(seg_source).astype(np.int64)
    seg_t = np.asarray(seg_target).astype(np.int64)

    # Host-side sharding prep (O(N*D), trivial next to the O(A*M*D) device work)
    # z inputs are L2-normalized (|z| <= 1), so fp16 quantization (~2.4e-4
    # rel) keeps the exp-sum well within fp32 reference noise while halving
    # the DMA volume.
    za = np.concatenate([zt, zs], axis=0)            # [A, D]
    zaT = np.ascontiguousarray(za.T.astype(np.float16))   # [D, A] fp16
    counts = np.bincount(seg_s, minlength=G).astype(np.float32)
    S = np.zeros((G, D), np.float32)
    np.add.at(S, seg_s, zs)
    v = S[seg_t] / (counts[seg_t] * np.float32(TEMPERATURE))[:, None]  # [M, D]
    vT = np.ascontiguousarray(v.T)                   # [D, M]

    in_maps = []
    for c in range(NCORES):
        j0 = c * MLOC
        in_maps.append({
            "zaT": zaT,
            "ztT": np.ascontiguousarray(zaT[:, j0:j0 + MLOC]),
            "vtT": np.ascontiguousarray(vT[:, j0:j0 + MLOC]),
        })

    nc = _get_nc()
    out = run_bass_kernel_spmd(nc, in_maps, core_ids=list(range(NCORES)))
    results = out.results

    # Host finish (float64): den = sum exp - exp(self); loss = sum log(den) - num.
    # The self dot replicates the device matmul bit-closely: fp16 inputs make
    # each product exact in fp32, and np.sum's fp32 pairwise accumulation
    # lands within ~2 ulp of the PE's accumulator (verified on hardware).
    h = zaT[:, :M].astype(np.float32)                # quantized z_target, [D, M]
    self_dot = np.sum(h * h, axis=0, dtype=np.float32).astype(np.float64)
    loss = 0.0
    for c in range(NCORES):
        r = results[c]["res"].astype(np.float64)     # [128, NSUM]
        r2 = results[c]["res2"].astype(np.float64)   # [128, 1]
        colsum = np.zeros((128, NJB))
        for jb in range(NJB):
            for g in range(NGRP):
                if (jb, g) in _DVE_COLS:
                    c0, n = _DVE_COLS[(jb, g)]
                    colsum[:, jb] += r2[:, c0:c0 + n].sum(axis=1)
                else:
                    c0, n = _ACC_COLS[(jb, g)]
                    colsum[:, jb] += r[:, c0:c0 + n].sum(axis=1)
        jj = c * MLOC + np.arange(NJB)[None, :] * 128 + np.arange(128)[:, None]
        den = colsum - np.exp(self_dot[jj] / TEMPERATURE)
        loss += np.sum(np.log(den))
        loss -= r2[:, 0].sum()
    return np.asarray(loss, dtype=np.float32)



# revision 19
# speedup vs baseline: 1.2157x; 1.2157x over previous
"""Trainium2 Bass kernel for a grouped contrastive loss (v4).

Math (matches the reference):
    den[j] = sum_{a != j} exp((z_a . z_tj) / T),  z_a = [z_target; z_source]
    num[j] = mean_{s in group(j)} (z_s . z_tj) / T      (exact linearity)
    loss   = sum_j log(den[j]) - num[j]

The z_t x z_t part of den is symmetric: each unordered 128x128 tile pair is
computed ONCE. A circulant orientation of the 32-tile-column graph (tile u
feeds column t iff (t-u) mod 32 in 1..15, plus u = t-16 for t >= 16, plus
the diagonal) gives every column tile in-degree 16 or 17; cores take columns
{c, 15-c, 16+c, 31-c} so every core gets bands of 16,16,17,17 tiles — the
SAME program shape on all 8 cores, with the host permuting za per core.
This cuts ACT exp work 24% (32768 -> 24832 elems/lane per core) — ACT is
the only exp-capable engine and the bottleneck.

Pipeline per chunk (<=2048 cols): PE matmuls (fp16, f32 psum) -> ACT exp
(scale=1/T) to fp16 SBUF -> DVE fold-fold-reduce column sums (fp16
TensorTensor adds run in 2x DVE mode; plain f32 TensorReduce does not).
PSUM is one [128,2048] tag double-buffered (8 banks) so matmul refill
always overlaps the previous chunk's exp. Column sums of the transposed
band halves are tile ROW sums: Pool (GpSimd) partition_all_reduce over the
exp'd band ranges, shipped out as [1, W] strips — Pool is otherwise idle.
The diagonal is masked on-device by accumulating a -2000*I matmul onto the
diag psum range before exp (exp -> 0): no self-term cancellation on the
host and fp16 exp outputs cannot overflow. num is computed on the host.

The input is ONE tensor [-2000I | I | ones | zt | band0 | z_s | band1 |
band2 | band3] (z_s stored once; chunks gather scattered za ranges),
streamed as pieces split between the SP HWDGE queue and the Pool SWDGE
queue so neither the per-DMA issue cost nor the serial transfer chain gates
the exp stream. The last two chunks use ACT accum_out; all other results
are DMA'd out early, so the tail is one accumulator read + a 2-column DMA.
"""

import numpy as np

TEMPERATURE = 0.07
N = 4096
M = 4096
D = 128
G = 64
NCORES = 8
ZS = 4096
BW = [2048, 2048, 2176, 2176]        # band cols per slot (16,16,17,17 tiles)

# za column layout (single input tensor per core). zt slot0 sits right
# before band0 so one DMA piece delivers both lhsT and the first rhs.
NEG, EYE, ONE, ZT123, ZT0 = 0, 128, 256, 384, 768
B0, ZSB, B1, B2, B3 = 896, 2944, 7040, 9088, 11264
ZA_COLS = 13440
_BB = [B0, B1, B2, B3]

# chunks: (slot, [(za_lo, za_hi, is_diag)...]); all use one 2048-wide psum
# tag, double-buffered. Slot 0 ramps up small while the DMA stream fills.
# The diag tile is the LAST band tile: cols 1920:2048 for slots 0/1 (16
# tiles), cols 2048:2176 for slots 2/3 (17 tiles).
CHUNKS = [
    (0, [(B0, B0 + 512, 0)]),
    (0, [(B0 + 512, B0 + 1536, 0)]),
    (0, [(B0 + 1536, B0 + 1920, 0), (ZSB, ZSB + 1152, 0)]),
    (0, [(ZSB + 1152, ZSB + 3072, 0)]),
    (0, [(B0 + 1920, B0 + 2048, 1), (ZSB + 3072, ZSB + 4096, 0)]),

    (1, [(ZSB, ZSB + 2048, 0)]),
    (1, [(ZSB + 2048, ZSB + 4096, 0)]),
    (1, [(B1, B1 + 1920, 0), (B1 + 1920, B1 + 2048, 1)]),

    # ALL bands run before the z_s rereads so every Pool all-reduce and
    # res2 strip DMA hides mid-stream instead of trailing the kernel tail;
    # the last two chunks are pure z_s with ACT accum_out
    (2, [(B2, B2 + 1024, 0)]),
    (2, [(B2 + 1024, B2 + 2048, 0), (B2 + 2048, B2 + 2176, 1)]),
    (3, [(B3, B3 + 2048, 0)]),
    (3, [(B3 + 2048, B3 + 2176, 1), (ZSB, ZSB + 1920, 0)]),

    (2, [(ZSB, ZSB + 2048, 0)]),
    (2, [(ZSB + 2048, ZSB + 4096, 0)]),
    (3, [(ZSB + 1920, ZSB + 3968, 0)]),
    (3, [(ZSB + 3968, ZSB + 4096, 0)]),
]
NCHUNK = len(CHUNKS)                 # 16
ACCUM_CHUNKS = {14: 14, 15: 15}      # chunk -> res col (reduces use cols 0..13)
RES_COLS = 16
RS2_COLS = 8448                      # [b0 2048 | b1 2048 | b2 2176 | b3 2176]
_RS2_BASE = [0, 2048, 4096, 6272]

# DMA pieces per queue, in issue order (SP = HWDGE, Pool = SWDGE)
DMA_SP = [(ZT0, B0 + 512), (B0 + 512, B0 + 1536), (ZSB, ZSB + 1152),
          (ZSB + 1152, ZSB + 3072), (ZSB + 3072, ZSB + 4096),
          (B1, B1 + 2048), (B3, B3 + 2176)]
DMA_POOL = [(B0 + 1536, B0 + 2048), (0, ZT123), (ZT123, ZT0),
            (B2, B2 + 2176)]


def tset(c):
    return [c, 15 - c, 16 + c, 31 - c]


def band(t):
    nb = []
    if t >= 16:
        nb.append(t - 16)
    nb += [(t - 15 + m) % 32 for m in range(15)]
    return nb + [t]                  # diagonal tile last


def _band_ranges(k, pieces):
    """[(chunk_off, width, res2_col)] for non-diag band cols in this chunk.
    Diag tiles are skipped: the host never reads their rowsums."""
    out = []
    off = 0
    b = _BB[k]
    for lo, hi, d in pieces:
        if not d and b <= lo and hi <= b + BW[k]:
            out.append((off, hi - lo, _RS2_BASE[k] + lo - b))
        off += hi - lo
    return out


_CACHE = {}


def _build_bass():
    import concourse.mybir as mybir
    from concourse import bacc
    from concourse import bass_isa
    from concourse.tile import TileContext

    f32 = mybir.dt.float32
    f16 = mybir.dt.float16
    Exp = mybir.ActivationFunctionType.Exp

    nc = bacc.Bacc("TRN2", num_devices=NCORES)
    za = nc.dram_tensor("za", [D, ZA_COLS], f16, kind="ExternalInput")
    res = nc.dram_tensor("res", [128, RES_COLS], f32, kind="ExternalOutput")
    res2 = nc.dram_tensor("res2", [1, RS2_COLS], f32, kind="ExternalOutput")

    with TileContext(nc) as tc:
        with (
            tc.tile_pool(name="persist", bufs=1) as persist,
            tc.tile_pool(name="scr", bufs=5) as scr_pool,
            tc.tile_pool(name="folds", bufs=2) as fold_pool,
            tc.tile_pool(name="prout", bufs=2) as pr_pool,
            tc.tile_pool(name="psmain", bufs=2, space="PSUM") as psum_pool,
        ):
            za_tile = persist.tile([128, ZA_COLS], f16, tag="za")
            for lo, hi in DMA_SP:
                nc.sync.dma_start(out=za_tile[:, lo:hi], in_=za[:, lo:hi])
            for lo, hi in DMA_POOL:
                nc.gpsimd.dma_start(out=za_tile[:, lo:hi], in_=za[:, lo:hi])
            res_tile = persist.tile([128, RES_COLS], f32, tag="res")

            neg_i = za_tile[:, NEG:NEG + 128]

            for ci, (k, pieces) in enumerate(CHUNKS):
                zk = ZT0 if k == 0 else ZT123 + (k - 1) * 128
                lhsT = za_tile[:, zk:zk + 128]
                w = sum(hi - lo for lo, hi, _ in pieces)
                ps = psum_pool.tile([128, 2048], f32, tag="ps")
                o = 0
                for lo, hi, is_diag in pieces:
                    p = lo
                    while p < hi:
                        # a matmul output must stay inside one 512-f32 psum
                        # bank: split at the 512-grid of the chunk offset
                        pe = min(p + 512 - o % 512, hi)
                        nc.tensor.matmul(
                            ps[:, o:o + pe - p], lhsT, za_tile[:, p:pe],
                            start=True, stop=True,
                        )
                        if is_diag:
                            # mask the self-similarity diagonal before exp
                            # (DVE add is deterministically ordered between
                            # the matmul write and the ACT read)
                            nc.vector.tensor_add(
                                out=ps[:, o:o + pe - p],
                                in0=ps[:, o:o + pe - p], in1=neg_i)
                        o += pe - p
                        p = pe
                scr = scr_pool.tile([128, 2048], f16, tag="scr")
                acc_col = ACCUM_CHUNKS.get(ci)
                nc.scalar.activation(
                    out=scr[:, 0:w], in_=ps[:, 0:w],
                    func=Exp, scale=1.0 / TEMPERATURE,
                    accum_out=None if acc_col is None
                    else res_tile[:, acc_col:acc_col + 1],
                )
                if acc_col is None:
                    if w >= 512:
                        h2, q = w // 2, w // 4
                        f1 = fold_pool.tile([128, 1024], f16, tag="f1")
                        f2 = fold_pool.tile([128, 512], f16, tag="f2")
                        nc.vector.tensor_add(
                            out=f1[:, :h2], in0=scr[:, 0:h2], in1=scr[:, h2:w])
                        nc.vector.tensor_add(
                            out=f2[:, :q], in0=f1[:, 0:q], in1=f1[:, q:h2])
                        red_in = f2[:, :q]
                    else:
                        red_in = scr[:, 0:w]
                    nc.vector.tensor_reduce(
                        out=res_tile[:, ci:ci + 1], in_=red_in,
                        axis=mybir.AxisListType.X, op=mybir.AluOpType.add)
                # transposed-half contributions: partition sums of the band
                # ranges on Pool, shipped as [1, W] strips
                for off, bw_, col in _band_ranges(k, pieces):
                    pr = pr_pool.tile([128, 2048], f32, tag="pr")
                    nc.gpsimd.partition_all_reduce(
                        pr[:, 0:bw_], scr[:, off:off + bw_],
                        channels=128, reduce_op=bass_isa.ReduceOp.add)
                    nc.sync.dma_start(
                        out=res2[0:1, col:col + bw_], in_=pr[0:1, 0:bw_])
                if ci == 13:
                    nc.sync.dma_start(out=res[:, 0:14], in_=res_tile[:, 0:14])
            nc.sync.dma_start(out=res[:, 14:16], in_=res_tile[:, 14:16])
    nc.compile()
    return nc


def _get_nc():
    if "nc" not in _CACHE:
        _CACHE["nc"] = _build_bass()
    return _CACHE["nc"]


def make_inmaps(z_source, z_target):
    """Host-side sharding: per-core za = [-2000I | I | ones | zt | bands/zs]."""
    zs = np.ascontiguousarray(z_source, dtype=np.float32)
    zt = np.ascontiguousarray(z_target, dtype=np.float32)
    za16 = np.concatenate([zt, zs], axis=0).astype(np.float16)   # [8192, D]
    za16T = np.ascontiguousarray(za16.T)                         # [D, 8192]
    eye = np.eye(128, dtype=np.float16)
    in_maps = []
    for c in range(NCORES):
        zac = np.zeros((D, ZA_COLS), np.float16)
        zac[:, NEG:NEG + 128] = -2000.0 * eye
        zac[:, EYE:EYE + 128] = eye
        zac[:, ONE] = 1.0
        zac[:, ZSB:ZSB + ZS] = za16T[:, 4096:8192]
        for k, t in enumerate(tset(c)):
            zk = ZT0 if k == 0 else ZT123 + (k - 1) * 128
            zac[:, zk:zk + 128] = za16T[:, 128 * t:128 * t + 128]
            cols = np.concatenate(
                [np.arange(128 * u, 128 * u + 128) for u in band(t)])
            zac[:, _BB[k]:_BB[k] + BW[k]] = za16T[:, cols]
        in_maps.append({"za": zac})
    return in_maps


def kernel(z_source, z_target, seg_source, seg_target):
    from concourse.bass_utils import run_bass_kernel_spmd

    zs = np.ascontiguousarray(z_source, dtype=np.float32)
    zt = np.ascontiguousarray(z_target, dtype=np.float32)
    seg_s = np.asarray(seg_source).astype(np.int64)
    seg_t = np.asarray(seg_target).astype(np.int64)

    in_maps = make_inmaps(zs, zt)
    nc = _get_nc()
    out = run_bass_kernel_spmd(nc, in_maps, core_ids=list(range(NCORES)))
    results = out.results

    slot_cols = [[] for _ in range(4)]
    for ci, (k, _p) in enumerate(CHUNKS):
        slot_cols[k].append(ACCUM_CHUNKS.get(ci, ci))

    den = np.zeros(M, np.float64)
    for c in range(NCORES):
        r = results[c]["res"].astype(np.float64)     # [128, 16]
        r2 = results[c]["res2"].astype(np.float64)[0]  # [8448]
        for k, t in enumerate(tset(c)):
            den[128 * t:128 * t + 128] += r[:, slot_cols[k]].sum(axis=1)
            bt = band(t)
            for i, u in enumerate(bt[:-1]):          # skip the diag tile
                den[128 * u:128 * u + 128] += \
                    r2[_RS2_BASE[k] + 128 * i:_RS2_BASE[k] + 128 * i + 128]

    # num on host in f64 (exact group-mean linearity)
    counts = np.bincount(seg_s, minlength=G).astype(np.float64)
    S = np.zeros((G, D), np.float64)
    np.add.at(S, seg_s, zs.astype(np.float64))
    v = S[seg_t] / (counts[seg_t] * TEMPERATURE)[:, None]
    num = np.einsum("md,md->m", v, zt.astype(np.float64))

    loss = np.sum(np.log(den)) - np.sum(num)
    return np.asarray(loss, dtype=np.float32)


# revision 23
# speedup vs baseline: 1.2451x; 1.0242x over previous
"""Trainium2 Bass kernel for a grouped contrastive loss (v4).

Math (matches the reference):
    den[j] = sum_{a != j} exp((z_a . z_tj) / T),  z_a = [z_target; z_source]
    num[j] = mean_{s in group(j)} (z_s . z_tj) / T      (exact linearity)
    loss   = sum_j log(den[j]) - num[j]

The z_t x z_t part of den is symmetric: each unordered 128x128 tile pair is
computed ONCE. A circulant orientation of the 32-tile-column graph (tile u
feeds column t iff (t-u) mod 32 in 1..15, plus u = t-16 for t >= 16, plus
the diagonal) gives every column tile in-degree 16 or 17; cores take columns
{c, 15-c, 16+c, 31-c} so every core gets bands of 16,16,17,17 tiles — the
SAME program shape on all 8 cores, with the host permuting za per core.
This cuts ACT exp work 24% (32768 -> 24832 elems/lane per core) — ACT is
the only exp-capable engine and the bottleneck.

Pipeline per chunk (<=2048 cols): PE matmuls (fp16, f32 psum) -> ACT exp
(scale=1/T) to fp16 SBUF -> DVE fold-fold-reduce column sums (fp16
TensorTensor adds run in 2x DVE mode; plain f32 TensorReduce does not).
PSUM is one [128,2048] tag double-buffered (8 banks) so matmul refill
always overlaps the previous chunk's exp. Column sums of the transposed
band halves are tile ROW sums: Pool (GpSimd) partition_all_reduce over the
exp'd band ranges, shipped out as [1, W] strips — Pool is otherwise idle.
The diagonal is masked on-device by accumulating a -2000*I matmul onto the
diag psum range before exp (exp -> 0): no self-term cancellation on the
host and fp16 exp outputs cannot overflow. num is computed on the host.

The input is ONE tensor [-2000I | I | ones | zt | band0 | z_s | band1 |
band2 | band3] (z_s stored once; chunks gather scattered za ranges),
streamed as pieces split between the SP HWDGE queue and the Pool SWDGE
queue so neither the per-DMA issue cost nor the serial transfer chain gates
the exp stream. The last two chunks use ACT accum_out; all other results
are DMA'd out early, so the tail is one accumulator read + a 2-column DMA.
"""

import numpy as np

TEMPERATURE = 0.07
N = 4096
M = 4096
D = 128
G = 64
NCORES = 8
ZS = 4096
BW = [2048, 2048, 2176, 2176]        # band cols per slot (16,16,17,17 tiles)

# za column layout (single input tensor per core). zt slot0 sits right
# before band0 so one DMA piece delivers both lhsT and the first rhs.
NEG, EYE, ONE, ZT123, ZT0 = 0, 128, 256, 384, 768
B0, ZSB, B1, B2, B3 = 896, 2944, 7040, 9088, 11264
ZA_COLS = 13440
_BB = [B0, B1, B2, B3]

# chunks: (slot, [(za_lo, za_hi, is_diag)...]); all use one 2048-wide psum
# tag, double-buffered. Slot 0 ramps up small while the DMA stream fills.
# The diag tile is the LAST band tile: cols 1920:2048 for slots 0/1 (16
# tiles), cols 2048:2176 for slots 2/3 (17 tiles).
CHUNKS = [
    (0, [(B0, B0 + 512, 0)]),
    (0, [(B0 + 512, B0 + 1536, 0)]),
    (0, [(B0 + 1536, B0 + 1920, 0), (ZSB, ZSB + 1152, 0)]),
    (0, [(ZSB + 1152, ZSB + 3072, 0)]),
    (0, [(ZSB + 3072, ZSB + 4096, 0), (B0 + 1920, B0 + 2048, 1)]),

    (1, [(ZSB, ZSB + 2048, 0)]),
    (1, [(ZSB + 2048, ZSB + 4096, 0)]),
    (1, [(B1, B1 + 1920, 0), (B1 + 1920, B1 + 2048, 1)]),

    # ALL bands run before the z_s rereads so every Pool all-reduce and
    # res2 strip DMA hides mid-stream instead of trailing the kernel tail;
    # the last two chunks are pure z_s with ACT accum_out
    (2, [(B2, B2 + 1024, 0)]),
    (2, [(B2 + 1024, B2 + 2048, 0), (ZSB, ZSB + 1024, 0)]),
    (3, [(B3, B3 + 2048, 0)]),
    (3, [(ZSB, ZSB + 1920, 0), (B3 + 2048, B3 + 2176, 1)]),

    (2, [(ZSB + 1024, ZSB + 2944, 0), (B2 + 2048, B2 + 2176, 1)]),
    (2, [(ZSB + 2944, ZSB + 4096, 0)]),
    (3, [(ZSB + 1920, ZSB + 3968, 0)]),
    (3, [(ZSB + 3968, ZSB + 4096, 0)]),
]
NCHUNK = len(CHUNKS)                 # 16
ACCUM_CHUNKS = {14: 14, 15: 15}      # chunk -> res col (reduces use cols 0..13)
RES_COLS = 16
RS2_COLS = 8448                      # [b0 2048 | b1 2048 | b2 2176 | b3 2176]
_RS2_BASE = [0, 2048, 4096, 6272]

# DMA pieces per queue, in issue order (SP = HWDGE, Pool = SWDGE)
DMA_SP = [(ZT0, B0 + 512), (B0 + 512, B0 + 1536), (ZSB, ZSB + 1152),
          (ZSB + 1152, ZSB + 3072), (ZSB + 3072, ZSB + 4096),
          (B1, B1 + 2048), (B3, B3 + 2176)]
DMA_POOL = [(B0 + 1536, B0 + 2048), (0, ZT123), (ZT123, ZT0),
            (B2, B2 + 2176)]


def tset(c):
    return [c, 15 - c, 16 + c, 31 - c]


def band(t):
    nb = []
    if t >= 16:
        nb.append(t - 16)
    nb += [(t - 15 + m) % 32 for m in range(15)]
    return nb + [t]                  # diagonal tile last


def _band_ranges(k, pieces):
    """[(chunk_off, width, res2_col)] for non-diag band cols in this chunk.
    Diag tiles are skipped: the host never reads their rowsums."""
    out = []
    off = 0
    b = _BB[k]
    for lo, hi, d in pieces:
        if not d and b <= lo and hi <= b + BW[k]:
            out.append((off, hi - lo, _RS2_BASE[k] + lo - b))
        off += hi - lo
    return out


_CACHE = {}


def _build_bass():
    import concourse.mybir as mybir
    from concourse import bacc
    from concourse import bass_isa
    from concourse.tile import TileContext

    f32 = mybir.dt.float32
    f16 = mybir.dt.float16
    Exp = mybir.ActivationFunctionType.Exp

    nc = bacc.Bacc("TRN2", num_devices=NCORES)
    za = nc.dram_tensor("za", [D, ZA_COLS], f16, kind="ExternalInput")
    res = nc.dram_tensor("res", [128, RES_COLS], f32, kind="ExternalOutput")
    res2 = nc.dram_tensor("res2", [1, RS2_COLS], f32, kind="ExternalOutput")

    with TileContext(nc) as tc:
        with (
            tc.tile_pool(name="persist", bufs=1) as persist,
            tc.tile_pool(name="scr", bufs=5) as scr_pool,
            tc.tile_pool(name="folds", bufs=2) as fold_pool,
            tc.tile_pool(name="prout", bufs=2) as pr_pool,
            tc.tile_pool(name="psmain", bufs=2, space="PSUM") as psum_pool,
        ):
            za_tile = persist.tile([128, ZA_COLS], f16, tag="za")
            for lo, hi in DMA_SP:
                nc.sync.dma_start(out=za_tile[:, lo:hi], in_=za[:, lo:hi])
            for lo, hi in DMA_POOL:
                nc.gpsimd.dma_start(out=za_tile[:, lo:hi], in_=za[:, lo:hi])
            res_tile = persist.tile([128, RES_COLS], f32, tag="res")

            neg_i = za_tile[:, NEG:NEG + 128]

            pending_fold = None
            for ci, (k, pieces) in enumerate(CHUNKS):
                zk = ZT0 if k == 0 else ZT123 + (k - 1) * 128
                lhsT = za_tile[:, zk:zk + 128]
                w = sum(hi - lo for lo, hi, _ in pieces)
                ps = psum_pool.tile([128, 2048], f32, tag="ps")
                o = 0
                for lo, hi, is_diag in pieces:
                    p = lo
                    while p < hi:
                        # a matmul output must stay inside one 512-f32 psum
                        # bank: split at the 512-grid of the chunk offset
                        pe = min(p + 512 - o % 512, hi)
                        nc.tensor.matmul(
                            ps[:, o:o + pe - p], lhsT, za_tile[:, p:pe],
                            start=True, stop=True,
                        )
                        if is_diag:
                            # mask the self-similarity diagonal before exp
                            # (DVE add is deterministically ordered between
                            # the matmul write and the ACT read)
                            nc.vector.tensor_add(
                                out=ps[:, o:o + pe - p],
                                in0=ps[:, o:o + pe - p], in1=neg_i)
                        o += pe - p
                        p = pe
                scr = scr_pool.tile([128, 2048], f16, tag="scr")
                acc_col = ACCUM_CHUNKS.get(ci)
                nc.scalar.activation(
                    out=scr[:, 0:w], in_=ps[:, 0:w],
                    func=Exp, scale=1.0 / TEMPERATURE,
                    accum_out=None if acc_col is None
                    else res_tile[:, acc_col:acc_col + 1],
                )

                def emit_folds(ci_, w_, scr_):
                    if w_ >= 512:
                        h2, q = w_ // 2, w_ // 4
                        f1 = fold_pool.tile([128, 1024], f16, tag="f1")
                        f2 = fold_pool.tile([128, 512], f16, tag="f2")
                        nc.vector.tensor_add(
                            out=f1[:, :h2], in0=scr_[:, 0:h2],
                            in1=scr_[:, h2:w_])
                        nc.vector.tensor_add(
                            out=f2[:, :q], in0=f1[:, 0:q], in1=f1[:, q:h2])
                        red_in = f2[:, :q]
                    else:
                        red_in = scr_[:, 0:w_]
                    nc.vector.tensor_reduce(
                        out=res_tile[:, ci_:ci_ + 1], in_=red_in,
                        axis=mybir.AxisListType.X, op=mybir.AluOpType.add)

                # DVE folds are deferred by one chunk so the next chunk's
                # diag-mask add sits ahead of them in DVE's in-order queue
                # (the mask gates ACT; folds have a chunk of slack)
                if pending_fold is not None:
                    emit_folds(*pending_fold)
                pending_fold = (ci, w, scr) if acc_col is None else None
                # transposed-half contributions: partition sums of the band
                # ranges on Pool, shipped as [1, W] strips
                for off, bw_, col in _band_ranges(k, pieces):
                    pr = pr_pool.tile([128, 2048], f32, tag="pr")
                    nc.gpsimd.partition_all_reduce(
                        pr[:, 0:bw_], scr[:, off:off + bw_],
                        channels=128, reduce_op=bass_isa.ReduceOp.add)
                    nc.sync.dma_start(
                        out=res2[0:1, col:col + bw_], in_=pr[0:1, 0:bw_])
                if ci == 14:
                    # all 14 reduce cols are final (ci13's folds flushed at
                    # the top of this block): ship them while ci14/15 stream
                    nc.sync.dma_start(out=res[:, 0:14], in_=res_tile[:, 0:14])
            assert pending_fold is None
            nc.sync.dma_start(out=res[:, 14:16], in_=res_tile[:, 14:16])
    nc.compile()
    return nc


def _get_nc():
    if "nc" not in _CACHE:
        _CACHE["nc"] = _build_bass()
    return _CACHE["nc"]


def make_inmaps(z_source, z_target):
    """Host-side sharding: per-core za = [-2000I | I | ones | zt | bands/zs]."""
    zs = np.ascontiguousarray(z_source, dtype=np.float32)
    zt = np.ascontiguousarray(z_target, dtype=np.float32)
    za16 = np.concatenate([zt, zs], axis=0).astype(np.float16)   # [8192, D]
    za16T = np.ascontiguousarray(za16.T)                         # [D, 8192]
    eye = np.eye(128, dtype=np.float16)
    in_maps = []
    for c in range(NCORES):
        zac = np.zeros((D, ZA_COLS), np.float16)
        zac[:, NEG:NEG + 128] = -2000.0 * eye
        zac[:, EYE:EYE + 128] = eye
        zac[:, ONE] = 1.0
        zac[:, ZSB:ZSB + ZS] = za16T[:, 4096:8192]
        for k, t in enumerate(tset(c)):
            zk = ZT0 if k == 0 else ZT123 + (k - 1) * 128
            zac[:, zk:zk + 128] = za16T[:, 128 * t:128 * t + 128]
            cols = np.concatenate(
                [np.arange(128 * u, 128 * u + 128) for u in band(t)])
            zac[:, _BB[k]:_BB[k] + BW[k]] = za16T[:, cols]
        in_maps.append({"za": zac})
    return in_maps


def kernel(z_source, z_target, seg_source, seg_target):
    from concourse.bass_utils import run_bass_kernel_spmd

    zs = np.ascontiguousarray(z_source, dtype=np.float32)
    zt = np.ascontiguousarray(z_target, dtype=np.float32)
    seg_s = np.asarray(seg_source).astype(np.int64)
    seg_t = np.asarray(seg_target).astype(np.int64)

    in_maps = make_inmaps(zs, zt)
    nc = _get_nc()
    out = run_bass_kernel_spmd(nc, in_maps, core_ids=list(range(NCORES)))
    results = out.results

    slot_cols = [[] for _ in range(4)]
    for ci, (k, _p) in enumerate(CHUNKS):
        slot_cols[k].append(ACCUM_CHUNKS.get(ci, ci))

    den = np.zeros(M, np.float64)
    for c in range(NCORES):
        r = results[c]["res"].astype(np.float64)     # [128, 16]
        r2 = results[c]["res2"].astype(np.float64)[0]  # [8448]
        for k, t in enumerate(tset(c)):
            den[128 * t:128 * t + 128] += r[:, slot_cols[k]].sum(axis=1)
            bt = band(t)
            for i, u in enumerate(bt[:-1]):          # skip the diag tile
                den[128 * u:128 * u + 128] += \
                    r2[_RS2_BASE[k] + 128 * i:_RS2_BASE[k] + 128 * i + 128]

    # num on host in f64 (exact group-mean linearity)
    counts = np.bincount(seg_s, minlength=G).astype(np.float64)
    S = np.zeros((G, D), np.float64)
    np.add.at(S, seg_s, zs.astype(np.float64))
    v = S[seg_t] / (counts[seg_t] * TEMPERATURE)[:, None]
    num = np.einsum("md,md->m", v, zt.astype(np.float64))

    loss = np.sum(np.log(den)) - np.sum(num)
    return np.asarray(loss, dtype=np.float32)
